# revision 11
# baseline (speedup 1.0000x reference)
"""Trainium2 Bass kernel for nn_AttentionModel_26723286516244.

Inception(9->512)x2 -> bidirectional Mamba x2 -> coordinate attention ->
trend attention -> FC.  Data-parallel: 16 samples over 8 NeuronCores.
Activations in (channels, T) layout (channel tiles of 128 partitions).
Matmuls in bf16 (fp32 PSUM accumulation); the selective scan runs on the
Vector engine (tensor_tensor_scan, bf16 operands, fp32 internal state).
"""

import numpy as np
import ml_dtypes

BF = ml_dtypes.bfloat16

B, T, CIN, FIL, OUT = 16, 512, 9, 512, 3
DM, DI, DS, DTR, KC = 512, 1024, 16, 32, 4
NH, HD = 8, 64
NCORE = 8
BL = B // NCORE
NT = FIL // 128      # 4
NTI = DI // 128      # 8
SL = 513             # scan slot (1 gap col + T)
HS = DS // 2         # 8 slots per scan half
HWD = HS * SL        # scan half-buffer width (4104)

NEG = -1.0e30

_cache = {}


def _f32(a):
    return np.ascontiguousarray(np.asarray(a, np.float32))


def _bf(a):
    return np.ascontiguousarray(np.asarray(a, np.float32).astype(BF))


def _prep_host(params):
    w = {}

    def inc_pack(p, name):
        w[f"{name}_p1"] = _bf(p['p1_w'][:, :, 0].T)
        w[f"{name}_p21"] = _bf(p['p21_w'][:, :, 0].T)
        w[f"{name}_p31"] = _bf(p['p31_w'][:, :, 0].T)
        w[f"{name}_p42"] = _bf(p['p42_w'][:, :, 0].T)
        w[f"{name}_p22"] = _bf(np.concatenate([p['p22_w'][:, :, j].T for j in range(3)], 0))
        w[f"{name}_p32"] = _bf(np.concatenate([p['p32_w'][:, :, j].T for j in range(5)], 0))
        for bn in ['p1_b', 'p21_b', 'p22_b', 'p31_b', 'p32_b', 'p42_b']:
            w[f"{name}_{bn}"] = _f32(p[bn][:, None])

    inc_pack(params['inc1'], 'i1')
    inc_pack(params['inc2'], 'i2')

    for li, bi in enumerate(['bi1', 'bi2']):
        p = params[bi]
        g, lb = _f32(p['ln_g']), _f32(p['ln_b'])
        for dk, dn in (('fwd', 'f'), ('bwd', 'b')):
            mp = p[dk]
            nm = f"m{li}{dn}"
            W1 = _f32(mp['in_proj_w']) * g[None, :]
            w[f"{nm}_w1"] = _bf(W1.T)                          # (512,2048)
            w[f"{nm}_c1"] = _f32((_f32(mp['in_proj_w']) @ lb)[:, None])
            dgw = _f32(mp['conv_w'][:, 0, :])
            dg = np.zeros((DI, KC * 128), np.float32)
            r = np.arange(128)
            for ti in range(NTI):
                for j in range(KC):
                    dg[ti * 128 + r, j * 128 + r] = dgw[ti * 128 + r, j]
            w[f"{nm}_dg"] = _bf(dg)
            w[f"{nm}_cb"] = _f32(mp['conv_b'][:, None])
            w[f"{nm}_xp"] = _bf(mp['x_proj_w'].T)              # (1024,64)
            w[f"{nm}_dtw"] = _bf(mp['dt_proj_w'].T)            # (32,1024)
            w[f"{nm}_dtb"] = _f32(mp['dt_proj_b'][:, None])
            w[f"{nm}_A"] = _f32(-np.exp(_f32(mp['A_log'])))    # (1024,16)
            w[f"{nm}_D"] = _f32(mp['D'][:, None])
            half = slice(0, DM) if dk == 'fwd' else slice(DM, 2 * DM)
            w[f"{nm}_wc"] = _bf((_f32(p['out_w'])[:, half] @ _f32(mp['out_proj_w'])).T)
        w[f"bi{li}_ob"] = _f32(p['out_b'][:, None])

    p = params['co']
    w['co_c1'] = _bf(p['conv1_w'].T)                           # (512,16)
    w['co_c1b'] = _f32(p['conv1_b'][:, None])
    sm = _f32(p['bn_g']) / np.sqrt(_f32(p['bn_var']) + 1e-5)
    bm = _f32(p['bn_b']) - _f32(p['bn_mean']) * sm
    w['co_sm'] = _f32(sm[:, None])
    w['co_bm'] = _f32(bm[:, None])
    w['co_wh'] = _bf(p['convh_w'].T)                           # (16,512)
    w['co_whb'] = _f32(p['convh_b'][:, None])
    w['co_ww'] = _bf(p['convw_w'].T)
    w['co_wwb'] = _f32(p['convw_b'][:, None])

    p = params['trend']
    for j in range(3):
        w[f"tr_cq{j}"] = _bf(p['convq_w'][0, j])               # (512,512) (I,O)
        w[f"tr_ck{j}"] = _bf(p['convk_w'][0, j])
    w['tr_cqb'] = _f32(p['convq_b'][:, None])
    w['tr_ckb'] = _f32(p['convk_b'][:, None])
    for nm2 in ['wq', 'wk', 'wv', 'wo']:
        w[f"tr_{nm2}"] = _bf(np.asarray(p[nm2]).T)
    for nm2 in ['bq', 'bk', 'bv', 'bo']:
        w[f"tr_{nm2}"] = _f32(p[nm2][:, None])
    w['tr_g'] = _f32(p['ln_g'][:, None])
    w['tr_b'] = _f32(p['ln_b'][:, None])

    fcw = _f32(params['fc_w']).reshape(OUT, T, FIL)
    w['fc_w'] = _bf(np.concatenate(
        [np.concatenate([fcw[o, :, i * 128:(i + 1) * 128].T for i in range(NT)], 0)
         for o in range(OUT)], 0))                             # (1536,512)
    w['fc_b'] = _f32(params['fc_b'][None, :])                  # (1,3)
    sel = np.zeros((64, 2 * DS * 128), np.float32)
    r = np.arange(128)
    for d in range(2 * DS):
        sel[DTR + d, d * 128 + r] = 1.0
    w['selbc'] = _bf(sel)                                      # (64, 4096)
    return w


def _build(host_w, debug_taps=()):
    import concourse.tile as tile
    from concourse import bacc, mybir
    from contextlib import ExitStack

    F32 = mybir.dt.float32
    BF16 = mybir.dt.bfloat16
    AF = mybir.ActivationFunctionType
    ALU = mybir.AluOpType
    AX = mybir.AxisListType

    nc = bacc.Bacc("TRN2", target_bir_lowering=False, debug=False,
                   enable_asserts=False)

    dram = {}
    for k_, v_ in host_w.items():
        dt = BF16 if v_.dtype == BF else F32
        dram[k_] = nc.dram_tensor(k_, list(v_.shape), dt, kind="ExternalInput")
    x_d = nc.dram_tensor("x", [BL, CIN, T + 4], BF16, kind="ExternalInput")
    out_d = nc.dram_tensor("out", [BL, OUT], F32, kind="ExternalOutput")
    taps = {}
    for tname, tshape in debug_taps:
        taps[tname] = nc.dram_tensor(tname, list(tshape), F32,
                                     kind="ExternalOutput")

    rank1A = all(np.allclose(host_w[f"m{li}{d}_A"],
                             np.broadcast_to(-np.arange(1, DS + 1, dtype=np.float32),
                                             (DI, DS)))
                 for li in range(2) for d in 'fb')

    with tile.TileContext(nc) as tc, ExitStack() as ctx:
        wv = ctx.enter_context(tc.tile_pool(name="wv", bufs=1))
        ws = ctx.enter_context(tc.tile_pool(name="ws", bufs=14))
        hp = ctx.enter_context(tc.tile_pool(name="hp", bufs=1))
        sp = ctx.enter_context(tc.tile_pool(name="sp", bufs=1))
        s2 = ctx.enter_context(tc.tile_pool(name="s2", bufs=2))
        s1 = ctx.enter_context(tc.tile_pool(name="s1", bufs=1))
        scp = ctx.enter_context(tc.tile_pool(name="scp", bufs=1))
        pm = ctx.enter_context(tc.tile_pool(name="pm", bufs=4, space="PSUM"))
        pm2 = ctx.enter_context(tc.tile_pool(name="pm2", bufs=2, space="PSUM"))
        pb = ctx.enter_context(tc.tile_pool(name="pb", bufs=2, space="PSUM"))

        V = {}
        for k_, a_ in host_w.items():
            if a_.dtype == BF:
                continue
            if a_.shape[0] <= 128 and a_.shape[1] <= 64:
                t_ = wv.tile(list(a_.shape), F32, tag=k_, name=k_)
                nc.sync.dma_start(t_[:], dram[k_].ap())
                V[k_] = t_
            elif a_.shape[1] <= 16:
                n = (a_.shape[0] + 127) // 128
                tl = []
                for i in range(n):
                    r0, r1 = i * 128, min((i + 1) * 128, a_.shape[0])
                    t_ = wv.tile([r1 - r0, a_.shape[1]], F32, tag=f"{k_}_{i}",
                                 name=f"{k_}_{i}")
                    nc.sync.dma_start(t_[:], dram[k_].ap()[r0:r1, :])
                    tl.append(t_)
                V[k_] = tl

        ones_c = wv.tile([128, 1], BF16, tag="ones_c")
        nc.gpsimd.memset(ones_c[:], 1.0)
        onesr_f = wv.tile([1, 128], F32, tag="onesr_f")
        nc.gpsimd.memset(onesr_f[:], 1.0)
        one_f = wv.tile([128, 1], F32, tag="one_f")
        nc.gpsimd.memset(one_f[:], 1.0)
        eps_t = wv.tile([1, 1], F32, tag="eps_t")
        nc.gpsimd.memset(eps_t[:], 1e-5)
        three_t = wv.tile([16, 1], F32, tag="three_t")
        nc.gpsimd.memset(three_t[:], 3.0)

        def wt(name, r0, rows, c0, cols):
            t_ = ws.tile([128, 512], BF16, tag="w", name=f"w_{name}_{r0}_{c0}")
            nc.sync.dma_start(t_[:rows, :cols],
                              dram[name].ap()[r0:r0 + rows, c0:c0 + cols])
            return t_

        def mm(ps, lhsT, rhs, start, stop):
            nc.tensor.matmul(ps, lhsT, rhs, start=start, stop=stop)

        def act(out_ap, in_ap, func, bias=0.0, scale=1.0, accum=None):
            nc.scalar.activation(out_ap, in_ap, func, bias=bias, scale=scale,
                                 accum_out=accum)

        def st(pool, shape, dtype, tag):
            return pool.tile(shape, dtype, tag=tag, name=tag)

        def tap(name, tiles):
            if name in taps:
                ap_ = taps[name].ap()
                for i, t_ in enumerate(tiles):
                    f_ = st(s1, [t_.shape[0], t_.shape[1]], F32, "tapf")
                    nc.vector.tensor_copy(f_[:], t_[:])
                    nc.sync.dma_start(
                        ap_[i * t_.shape[0]:(i + 1) * t_.shape[0], :t_.shape[1]],
                        f_[:])

        # ---------------- inception ----------------
        def inception(xin, name):
            outs = []
            P = xin[0].shape[0]
            nk = len(xin)

            ps = st(pm, [128, T], F32, "pmA")
            for ki, xt_ in enumerate(xin):
                w_ = wt(f"{name}_p1", ki * 128, P, 0, 128)
                mm(ps[:], w_[:P, :128], xt_[:, 2:2 + T], ki == 0, ki == nk - 1)
            o1 = st(hp, [128, T], BF16, "inc_o1")
            act(o1[:], ps[:], AF.Gelu, bias=V[f"{name}_p1_b"][:])
            outs.append(o1)

            for br, kw, kk in (("2", "p21", 3), ("3", "p31", 5)):
                ps = st(pm, [128, T], F32, "pmA")
                for ki, xt_ in enumerate(xin):
                    w_ = wt(f"{name}_{kw}", ki * 128, P, 0, 128)
                    mm(ps[:], w_[:P, :128], xt_[:, 2:2 + T], ki == 0, ki == nk - 1)
                a_ = st(s1, [128, T + 4], BF16, f"inca{br}")
                nc.gpsimd.memset(a_[:, 0:2], 0.0)
                nc.gpsimd.memset(a_[:, T + 2:T + 4], 0.0)
                act(a_[:, 2:2 + T], ps[:], AF.Gelu, bias=V[f"{name}_{kw}_b"][:])
                ps = st(pm, [128, T], F32, "pmA")
                wname = f"{name}_p22" if kk == 3 else f"{name}_p32"
                off = 1 if kk == 3 else 0
                for j in range(kk):
                    w_ = wt(wname, j * 128, 128, 0, 128)
                    mm(ps[:], w_[:, :128], a_[:, off + j:off + j + T],
                       j == 0, j == kk - 1)
                o_ = st(hp, [128, T], BF16, f"inc_o{br}")
                bname = f"{name}_p22_b" if kk == 3 else f"{name}_p32_b"
                act(o_[:], ps[:], AF.Gelu, bias=V[bname][:])
                outs.append(o_)

            ps = st(pm, [128, T], F32, "pmA")
            for ki, xt_ in enumerate(xin):
                tmp = st(s1, [P, T], BF16, "incmpt")
                mp_ = st(s1, [P, T], BF16, "incmp")
                nc.vector.tensor_tensor(tmp[:], xt_[:, 1:1 + T], xt_[:, 2:2 + T],
                                        ALU.max)
                nc.vector.tensor_tensor(mp_[:], tmp[:], xt_[:, 3:3 + T], ALU.max)
                w_ = wt(f"{name}_p42", ki * 128, P, 0, 128)
                mm(ps[:], w_[:P, :128], mp_[:], ki == 0, ki == nk - 1)
            o4 = st(hp, [128, T], BF16, "inc_o4")
            act(o4[:], ps[:], AF.Gelu, bias=V[f"{name}_p42_b"][:])
            outs.append(o4)
            return outs

        def pad_neg(tiles, tagp):
            out = []
            for i, t_ in enumerate(tiles):
                pt = st(sp, [128, T + 4], BF16, f"{tagp}{i}")
                nc.gpsimd.memset(pt[:, 0:2], NEG)
                nc.gpsimd.memset(pt[:, T + 2:T + 4], NEG)
                nc.vector.tensor_copy(pt[:, 2:2 + T], t_[:])
                out.append(pt)
            return out

        # ---------------- layernorm over channels ----------------
        def ln_stats(tiles):
            pss = st(pb, [1, T], F32, "pbA")
            for i, t_ in enumerate(tiles):
                mm(pss[:], ones_c[:], t_[:], i == 0, i == NT - 1)
            psq = st(pb, [1, T], F32, "pbA")
            for i, t_ in enumerate(tiles):
                s_ = st(s1, [128, T], BF16, "lnsq")
                act(s_[:], t_[:], AF.Square)
                mm(psq[:], ones_c[:], s_[:], i == 0, i == NT - 1)
            mu = st(sp, [1, T], F32, "ln_mu")
            act(mu[:], pss[:], AF.Copy, scale=1.0 / FIL)
            ex2 = st(sp, [1, T], F32, "ln_ex2")
            act(ex2[:], psq[:], AF.Copy, scale=1.0 / FIL)
            mu2 = st(sp, [1, T], F32, "ln_mu2")
            act(mu2[:], mu[:], AF.Square)
            var = st(sp, [1, T], F32, "ln_var")
            nc.vector.tensor_tensor(var[:], ex2[:], mu2[:], ALU.subtract)
            std = st(sp, [1, T], F32, "ln_std")
            act(std[:], var[:], AF.Sqrt, bias=eps_t[:])
            inv = st(sp, [1, T], F32, "ln_inv")
            nc.vector.reciprocal(inv[:], std[:])
            mub = st(pb, [128, T], F32, "pbA")
            mm(mub[:], onesr_f[:], mu[:], True, True)
            invb = st(pb, [128, T], F32, "pbA")
            mm(invb[:], onesr_f[:], inv[:], True, True)
            return mub, invb

        def ln_apply(tiles, mub, invb, tagp, gt=None, bt=None, out_pool=None):
            pool = out_pool or sp
            outs = []
            for i, t_ in enumerate(tiles):
                d_ = st(s1, [128, T], F32, "ln_d")
                nc.vector.tensor_tensor(d_[:], t_[:], mub[:], ALU.subtract)
                o_ = st(pool, [128, T], BF16, f"{tagp}{i}")
                if gt is None:
                    nc.vector.tensor_tensor(o_[:], d_[:], invb[:], ALU.mult)
                else:
                    n_ = st(s1, [128, T], F32, "ln_n")
                    nc.vector.tensor_tensor(n_[:], d_[:], invb[:], ALU.mult)
                    act(o_[:], n_[:], AF.Identity, bias=bt[i][:], scale=gt[i][:])
                outs.append(o_)
            return outs

        # ---------------- mamba direction ----------------
        def mamba_dir(nm, xn, rev, sb, ot_ps):
            def rd(ap_):
                return ap_[:, ::-1] if rev else ap_

            xipad, sz = [], []
            c1t = V[f"{nm}_c1"]
            for cc in range(4):
                w1t = [wt(f"{nm}_w1", ki * 128, 128, cc * 512, 512)
                       for ki in range(4)]
                for mi in range(4):
                    mt = cc * 4 + mi
                    ps = st(pm2, [128, T], F32, "pmB")
                    for ki in range(4):
                        mm(ps[:], w1t[ki][:, mi * 128:(mi + 1) * 128],
                           rd(xn[ki][:]), ki == 0, ki == 3)
                    if mt < 8:
                        xp_ = st(sp, [128, T + 3], BF16, f"xip{mt}")
                        nc.gpsimd.memset(xp_[:, 0:3], 0.0)
                        act(xp_[:, 3:3 + T], ps[:], AF.Identity, bias=c1t[mt][:])
                        xipad.append(xp_)
                    else:
                        z_ = st(sp, [128, T], BF16, f"{sb}sz{mt - 8}")
                        act(z_[:], ps[:], AF.Silu, bias=c1t[mt][:])
                        sz.append(z_)

            xi2 = []
            for ti in range(NTI):
                dg_ = wt(f"{nm}_dg", ti * 128, 128, 0, 512)
                ps = st(pm2, [128, T], F32, "pmB")
                for j in range(KC):
                    mm(ps[:], dg_[:, j * 128:(j + 1) * 128],
                       xipad[ti][:, j:j + T], j == 0, j == KC - 1)
                x2_ = st(sp, [128, T], BF16, f"xi2_{ti}")
                act(x2_[:], ps[:], AF.Silu, bias=V[f"{nm}_cb"][ti][:])
                xi2.append(x2_)

            ps64 = st(pb, [64, T], F32, "pbA")
            for ti in range(NTI):
                xpw = wt(f"{nm}_xp", ti * 128, 128, 0, 64)
                mm(ps64[:], xpw[:, :64], xi2[ti][:], ti == 0, ti == NTI - 1)
            xdb = st(sp, [64, T], BF16, "xdb")
            act(xdb[:], ps64[:], AF.Copy)

            Bb = st(sp, [128, DS * SL], BF16, "Bb")
            Cb = st(sp, [128, DS * SL], BF16, "Cb")
            nc.gpsimd.memset(Bb[:, 0:DS * SL:SL], 0.0)
            nc.gpsimd.memset(Cb[:, 0:DS * SL:SL], 0.0)
            selt = [wt('selbc', 0, 64, i * 512, 512) for i in range(8)]
            for d in range(DS):
                s_i, s_c = divmod(d * 128, 512)
                psd = st(pb, [128, T], F32, "pbA")
                mm(psd[:], selt[s_i][:64, s_c:s_c + 128], xdb[:], True, True)
                act(Bb[:, d * SL + 1:(d + 1) * SL], psd[:], AF.Copy)
                s_i, s_c = divmod((DS + d) * 128, 512)
                psd = st(pb, [128, T], F32, "pbA")
                mm(psd[:], selt[s_i][:64, s_c:s_c + 128], xdb[:], True, True)
                nc.vector.tensor_copy(Cb[:, d * SL + 1:(d + 1) * SL], psd[:])

            Acols = V[f"{nm}_A"]
            wct = [wt(f"{nm}_wc", ki * 128, 128, 0, 512) for ki in range(NTI)]
            for ti in range(NTI):
                dtw_ = wt(f"{nm}_dtw", 0, DTR, ti * 128, 128)
                ps = st(pm2, [128, T], F32, "pmB")
                mm(ps[:], dtw_[:DTR, :128], xdb[0:DTR, :], True, True)
                e_ = st(s1, [128, T], F32, "sc_e")
                act(e_[:], ps[:], AF.Exp, bias=V[f"{nm}_dtb"][ti][:])
                dt_ = st(s2, [128, SL], BF16, "sc_dt")
                act(dt_[:, 1:SL], e_[:], AF.Ln, bias=one_f[:])
                u_ = st(s2, [128, SL], BF16, "sc_u")
                nc.gpsimd.memset(u_[:, 0:1], 0.0)
                nc.vector.tensor_tensor(u_[:, 1:SL], dt_[:, 1:SL], xi2[ti][:],
                                        ALU.mult)

                b1 = st(scp, [128, HWD], BF16, "sc_b1")
                b2 = st(scp, [128, HWD], BF16, "sc_b2")
                b3 = st(scp, [128, HWD], BF16, "sc_b3")
                ya = st(s2, [128, SL], BF16, "sc_ya")
                ub = u_[:].unsqueeze(1).broadcast_to([128, HS, SL])

                def half(dlo, DAb, DBb, Hb):
                    nc.gpsimd.memset(DBb[:, 0:HWD:SL], 0.0)
                    nc.vector.tensor_tensor(
                        DBb[:].rearrange("p (d l) -> p d l", l=SL), ub,
                        Bb[:, dlo * SL:(dlo + HS) * SL].rearrange(
                            "p (d l) -> p d l", l=SL), ALU.mult)
                    nc.vector.tensor_tensor_scan(Hb[:], DAb[:], DBb[:],
                                                 0.0, ALU.mult, ALU.add)
                    nc.vector.tensor_tensor(DBb[:], Hb[:],
                                            Cb[:, dlo * SL:(dlo + HS) * SL],
                                            ALU.mult)
                    nc.vector.tensor_tensor(Hb[:, 0:4 * SL], DBb[:, 0:4 * SL],
                                            DBb[:, 4 * SL:HWD], ALU.add)
                    nc.vector.tensor_tensor(DBb[:, 0:2 * SL], Hb[:, 0:2 * SL],
                                            Hb[:, 2 * SL:4 * SL], ALU.add)
                    nc.vector.tensor_tensor(ya[:], DBb[:, 0:SL],
                                            DBb[:, SL:2 * SL], ALU.add)

                nc.gpsimd.memset(b1[:, 0:HWD:SL], 0.0)
                for d in range(HS):
                    act(b1[:, d * SL + 1:(d + 1) * SL], dt_[:, 1:SL], AF.Exp,
                        scale=Acols[ti][:, d:d + 1])
                half(0, b1, b2, b3)
                y1 = st(s2, [128, SL], BF16, "sc_y1")
                nc.vector.tensor_copy(y1[:], ya[:])

                if rank1A:
                    q8 = b1[:, (HS - 1) * SL:HS * SL].unsqueeze(1) \
                        .broadcast_to([128, HS, SL])
                    nc.vector.tensor_tensor(
                        b3[:].rearrange("p (d l) -> p d l", l=SL),
                        b1[:].rearrange("p (d l) -> p d l", l=SL),
                        q8, ALU.mult)
                else:
                    nc.gpsimd.memset(b3[:, 0:HWD:SL], 0.0)
                    for d in range(HS):
                        act(b3[:, d * SL + 1:(d + 1) * SL], dt_[:, 1:SL],
                            AF.Exp, scale=Acols[ti][:, HS + d:HS + d + 1])
                half(HS, b3, b2, b1)

                dxi = st(s1, [128, T], BF16, "sc_dxi")
                nc.vector.tensor_scalar(dxi[:], xi2[ti][:], V[f"{nm}_D"][ti][:],
                                        None, ALU.mult)
                ysum = st(s1, [128, T], BF16, "sc_ys")
                nc.vector.tensor_tensor(ysum[:], y1[:, 1:SL], ya[:, 1:SL], ALU.add)
                y_ = st(s1, [128, T], BF16, "sc_y")
                nc.vector.tensor_tensor(y_[:], ysum[:], dxi[:], ALU.add)
                g_ = st(s1, [128, T], BF16, "sc_g")
                nc.vector.tensor_tensor(g_[:], y_[:], sz[ti][:], ALU.mult)
                for ot in range(NT):
                    mm(ot_ps[ot][:], wct[ti][:, ot * 128:(ot + 1) * 128],
                       rd(g_[:]), (not rev) and ti == 0, rev and ti == NTI - 1)

        # ---------------- bimamba ----------------
        def bimamba(li, h_tiles):
            mub, invb = ln_stats(h_tiles)
            xn = ln_apply(h_tiles, mub, invb, "i1p")
            ot_ps = [st(pm, [128, T], F32, "pmA") for _ in range(NT)]
            mamba_dir(f"m{li}f", xn, False, "f", ot_ps)
            mamba_dir(f"m{li}b", xn, True, "b", ot_ps)
            outs = []
            for ot in range(NT):
                s_ = st(s1, [128, T], F32, "bim_s")
                act(s_[:], ot_ps[ot][:], AF.Identity, bias=V[f"bi{li}_ob"][ot][:])
                o_ = st(hp, [128, T], BF16, f"h{li % 2}_{ot}")
                nc.vector.tensor_tensor(o_[:], s_[:], h_tiles[ot][:], ALU.add)
                outs.append(o_)
            return outs

        # ---------------- coordinate attention ----------------
        def coord(h_tiles):
            xw = st(sp, [128, NT], F32, "co_xw")
            for i, t_ in enumerate(h_tiles):
                c_ = st(s1, [128, T], BF16, "co_cp")
                act(c_[:], t_[:], AF.Copy, scale=1.0 / T, accum=xw[:, i:i + 1])
            xwb = st(sp, [128, NT], BF16, "co_xwb")
            nc.vector.tensor_copy(xwb[:], xw[:])
            c1w = [wt('co_c1', i * 128, 128, 0, 16) for i in range(NT)]
            psy = st(pb, [16, T], F32, "pbA")
            for i, t_ in enumerate(h_tiles):
                mm(psy[:], c1w[i][:, :16], t_[:], i == 0, i == NT - 1)
            yw1 = st(pb, [16, 1], F32, "pbA")
            for i in range(NT):
                mm(yw1[:], c1w[i][:, :16], xwb[:, i:i + 1], i == 0, i == NT - 1)

            def hswish(psv, width, tg):
                yp = st(sp, [16, width], F32, f"co_yp{tg}")
                act(yp[:], psv, AF.Identity, bias=V['co_bm'][:], scale=V['co_sm'][:])
                t_ = st(sp, [16, width], F32, f"co_t{tg}")
                act(t_[:], yp[:], AF.Relu, bias=three_t[:])
                t2 = st(sp, [16, width], F32, f"co_t2{tg}")
                nc.vector.tensor_scalar(t2[:], t_[:], 6.0, None, ALU.min)
                t3 = st(sp, [16, width], BF16, f"co_t3{tg}")
                act(t3[:], t2[:], AF.Copy, scale=1.0 / 6.0)
                ypb = st(sp, [16, width], BF16, f"co_ypb{tg}")
                nc.vector.tensor_copy(ypb[:], yp[:])
                hs = st(sp, [16, width], BF16, f"co_hs{tg}")
                nc.vector.tensor_tensor(hs[:], ypb[:], t3[:], ALU.mult)
                return hs

            hsh = hswish(psy[:], T, "h")
            hswv = hswish(yw1[:], 1, "w")
            whw = wt('co_wh', 0, 16, 0, 512)
            wwt = wt('co_ww', 0, 16, 0, 512)
            outs = []
            for ot in range(NT):
                psh = st(pm, [128, T], F32, "pmA")
                mm(psh[:], whw[:16, ot * 128:(ot + 1) * 128], hsh[:], True, True)
                ah = st(s1, [128, T], BF16, "co_ah")
                act(ah[:], psh[:], AF.Sigmoid, bias=V['co_whb'][ot][:])
                psw = st(pb, [128, 1], F32, "pbA")
                mm(psw[:], wwt[:16, ot * 128:(ot + 1) * 128], hswv[:], True, True)
                aw = st(s2, [128, 1], F32, "co_aw")
                act(aw[:], psw[:], AF.Sigmoid, bias=V['co_wwb'][ot][:])
                m1 = st(s1, [128, T], BF16, "co_m1")
                nc.vector.tensor_tensor(m1[:], h_tiles[ot][:], ah[:], ALU.mult)
                o_ = st(hp, [128, T], BF16, f"inc_o{ot + 1}")
                nc.vector.tensor_scalar(o_[:], m1[:], aw[:], None, ALU.mult)
                outs.append(o_)
            return outs

        # ---------------- trend attention ----------------
        def trend(h_tiles):
            hpad = []
            for i, t_ in enumerate(h_tiles):
                pt = st(sp, [128, T + 4], BF16, f"trp{i}")
                nc.gpsimd.memset(pt[:, 0:2], 0.0)
                nc.gpsimd.memset(pt[:, T + 2:T + 4], 0.0)
                nc.vector.tensor_copy(pt[:, 2:2 + T], t_[:])
                hpad.append(pt)

            def qk_proj(cw, cbn, wn, bn, ctag, ptag):
                cwt = [[wt(f"{cw}{j}", ki * 128, 128, 0, 512) for ki in range(NT)]
                       for j in range(3)]
                ctiles = []
                for ot in range(NT):
                    ps = st(pm, [128, T], F32, "pmA")
                    first = True
                    for j in range(3):
                        for ki in range(NT):
                            mm(ps[:], cwt[j][ki][:, ot * 128:(ot + 1) * 128],
                               hpad[ki][:, 1 + j:1 + j + T],
                               first, (j == 2 and ki == NT - 1))
                            first = False
                    c_ = st(sp, [128, T], BF16, f"{ctag}{ot}")
                    act(c_[:], ps[:], AF.Identity, bias=V[cbn][ot][:])
                    ctiles.append(c_)
                wwt_ = [wt(wn, ki * 128, 128, 0, 512) for ki in range(NT)]
                ptl = []
                for ot in range(NT):
                    ps = st(pm, [128, T], F32, "pmA")
                    for ki in range(NT):
                        mm(ps[:], wwt_[ki][:, ot * 128:(ot + 1) * 128],
                           ctiles[ki][:], ki == 0, ki == NT - 1)
                    p_ = st(sp, [128, T], BF16, f"{ptag}{ot}")
                    act(p_[:], ps[:], AF.Identity, bias=V[bn][ot][:])
                    ptl.append(p_)
                return ptl

            q = qk_proj("tr_cq", "tr_cqb", "tr_wq", "tr_bq", "xip", "fsz")
            kk = qk_proj("tr_ck", "tr_ckb", "tr_wk", "tr_bk", "i1p", "bsz")
            v = []
            wvt = [wt('tr_wv', ki * 128, 128, 0, 512) for ki in range(NT)]
            for ot in range(NT):
                ps = st(pm, [128, T], F32, "pmA")
                for ki in range(NT):
                    mm(ps[:], wvt[ki][:, ot * 128:(ot + 1) * 128],
                       h_tiles[ki][:], ki == 0, ki == NT - 1)
                v_ = st(sp, [128, T], BF16, f"xi2_{ot}")
                act(v_[:], ps[:], AF.Identity, bias=V['tr_bv'][ot][:])
                v.append(v_)

            o_sb = []
            pT = [st(sp, [128, T], BF16, f"xi2_{4 + kc2}") for kc2 in range(4)]
            vT = [st(sp, [128, 64], BF16, f"tr_vT{kc2}") for kc2 in range(4)]
            for til in range(NT):
                pso = st(pm, [128, T], F32, "pmA")
                for row in range(2):
                    r0 = row * 64
                    qhh = q[til][r0:r0 + 64, :]
                    khh = kk[til][r0:r0 + 64, :]
                    vhh = v[til][r0:r0 + 64, :]
                    for kc2 in range(4):
                        nc.sync.dma_start_transpose(
                            vT[kc2][:], vhh[:, kc2 * 128:(kc2 + 1) * 128])
                    for ch in range(4):
                        ps = st(pb, [128, T], F32, "pbA")
                        mm(ps[:], qhh[:, ch * 128:(ch + 1) * 128], khh[:],
                           True, True)
                        nmx = st(s2, [128, 1], F32, "tr_nmx")
                        nc.vector.tensor_reduce(nmx[:], ps[:], AX.X, ALU.max,
                                                negate=True)
                        nmx8 = st(s2, [128, 1], F32, "tr_nmx8")
                        act(nmx8[:], nmx[:], AF.Copy, scale=1.0 / 8.0)
                        pe_ = st(s1, [128, T], BF16, "tr_pe")
                        ssum = st(s2, [128, 1], F32, "tr_ss")
                        nc.scalar.activation(pe_[:], ps[:], AF.Exp, bias=nmx8[:],
                                             scale=1.0 / 8.0, accum_out=ssum[:])
                        rs = st(s2, [128, 1], F32, "tr_rs")
                        nc.vector.reciprocal(rs[:], ssum[:])
                        prb = st(s1, [128, T], BF16, "tr_prb")
                        nc.scalar.mul(prb[:], pe_[:], rs[:])
                        for kc2 in range(4):
                            nc.sync.dma_start_transpose(
                                pT[kc2][:, ch * 128:(ch + 1) * 128],
                                prb[:, kc2 * 128:(kc2 + 1) * 128])
                    for kc2 in range(4):
                        mm(pso[r0:r0 + 64, :], vT[kc2][:, :64], pT[kc2][:],
                           kc2 == 0, kc2 == 3)
                ot_ = st(sp, [128, T], BF16, f"tr_o{til}")
                act(ot_[:], pso[:], AF.Copy)
                o_sb.append(ot_)

            wot = [wt('tr_wo', ki * 128, 128, 0, 512) for ki in range(NT)]
            rt = []
            for ot in range(NT):
                ps = st(pm, [128, T], F32, "pmA")
                for ki in range(NT):
                    mm(ps[:], wot[ki][:, ot * 128:(ot + 1) * 128],
                       o_sb[ki][:], ki == 0, ki == NT - 1)
                s_ = st(s1, [128, T], F32, "tr_s")
                act(s_[:], ps[:], AF.Identity, bias=V['tr_bo'][ot][:])
                r_ = st(sp, [128, T], BF16, f"tr_r{ot}")
                nc.vector.tensor_tensor(r_[:], s_[:], h_tiles[ot][:], ALU.add)
                rt.append(r_)
            mub, invb = ln_stats(rt)
            return ln_apply(rt, mub, invb, "h0_", gt=V['tr_g'], bt=V['tr_b'],
                            out_pool=hp)

        # ---------------- per-sample main ----------------
        for s in range(BL):
            xt = st(sp, [CIN, T + 4], BF16, "x_in")
            nc.sync.dma_start(xt[:], x_d.ap()[s])
            h = inception([xt], 'i1')
            tap(f"tap_inc1_{s}", h)
            hp4 = pad_neg(h, "i1p")
            h = inception(hp4, 'i2')
            tap(f"tap_inc2_{s}", h)
            h = bimamba(0, h)
            tap(f"tap_bi1_{s}", h)
            h = bimamba(1, h)
            tap(f"tap_bi2_{s}", h)
            h = coord(h)
            tap(f"tap_co_{s}", h)
            h = trend(h)
            tap(f"tap_tr_{s}", h)

            acc_cols = st(sp, [128, OUT], F32, "fc_cols")
            for o in range(OUT):
                tmps = []
                for i in range(NT):
                    fcw_ = wt('fc_w', (o * NT + i) * 128, 128, 0, 512)
                    tmp = st(s2, [128, T], BF16, f"fc_tmp{i % 2}")
                    nc.vector.tensor_tensor(tmp[:], fcw_[:], h[i][:], ALU.mult)
                    tmps.append(tmp)
                t01 = st(s1, [128, T], BF16, "fc_t01")
                nc.vector.tensor_tensor(t01[:], tmps[0][:], tmps[1][:], ALU.add)
                t23 = st(s1, [128, T], BF16, "fc_t23")
                nc.vector.tensor_tensor(t23[:], tmps[2][:], tmps[3][:], ALU.add)
                acc = st(s1, [128, T], BF16, "fc_acc")
                nc.vector.tensor_tensor(acc[:], t01[:], t23[:], ALU.add)
                dum = st(s1, [128, T], BF16, "fc_dum")
                act(dum[:], acc[:], AF.Copy, accum=acc_cols[:, o:o + 1])
            accb = st(sp, [128, OUT], BF16, "fc_accb")
            nc.vector.tensor_copy(accb[:], acc_cols[:])
            psf = st(pb, [1, OUT], F32, "pbA")
            mm(psf[:], ones_c[:], accb[:], True, True)
            ocol = st(sp, [1, OUT], F32, "fc_out")
            nc.vector.tensor_tensor(ocol[:], psf[:], V['fc_b'][:], ALU.add)
            nc.sync.dma_start(out_d.ap()[s:s + 1, :], ocol[:])

    nc.compile()
    return nc


def _prep_x(x):
    xt = np.ascontiguousarray(np.transpose(np.asarray(x, np.float32), (0, 2, 1)))
    xp = np.zeros((B, CIN, T + 4), np.float32)
    xp[:, :, 2:2 + T] = xt
    xp[:, :, 1] = NEG
    xp[:, :, T + 2] = NEG
    return xp.astype(BF)


def kernel(x, params):
    if "built" not in _cache:
        host_w = _prep_host(params)
        nc = _build(host_w)
        _cache["built"] = (nc, host_w)
    nc, host_w = _cache["built"]

    from concourse.bass_utils import run_bass_kernel_spmd
    xp = _prep_x(x)
    in_maps = []
    for c in range(NCORE):
        im = dict(host_w)
        im["x"] = xp[c * BL:(c + 1) * BL]
        in_maps.append(im)
    res = run_bass_kernel_spmd(nc, in_maps, core_ids=list(range(NCORE)))
    out = np.concatenate([res.results[c]["out"] for c in range(NCORE)], 0)
    return out.astype(np.float32)
